# revision 1
# baseline (speedup 1.0000x reference)
"""Trainium2 Bass kernel for nn_LiquidModel (moe_routing).

Strategy:
 - The reference MoE routing is degenerate: top-2 experts are chosen from
   token 0's gate scores and applied to ALL tokens, and the two expert
   outputs are averaged.  mean_k(x @ W_k + b_k) == x @ mean(W_k) + mean(b_k),
   and row 0 of x evolves independently of other rows through the MoE stack,
   so the whole routing chain is computed on host (float64) and each MoE
   layer collapses to a single dense GEMM with pre-averaged weights.
 - Data-parallel over tokens: each of the 8 cores processes 512 tokens.
   Activations are kept feature-major (x^T: [feat, tok]) so that every dense
   GEMM uses the weight matrix [K=feat_in, M=feat_out] directly as the
   stationary operand and layer biases are per-partition ACT biases.
 - Attention requires full K/V; cores exchange K^T / V via two AllGather
   collectives, then each core runs exact softmax attention for its 512
   queries (scores are tiny, |S|<0.03, so exp without max-subtraction).
 - All matmuls run in fp32r (TF32-like, full PE rate at free-dim >= 256).
"""
import ml_dtypes
import numpy as np

import concourse.bacc as bacc
import concourse.bass as bass
import concourse.mybir as mybir
import concourse.tile as tile
from concourse import bass_utils

FP32 = mybir.dt.float32
FP32R = mybir.dt.float32r
BF16 = mybir.dt.bfloat16
AF = mybir.ActivationFunctionType
ALU = mybir.AluOpType

NCORES = 8
N, D, DFF, H, L = 4096, 1024, 2048, 4, 3
TOK = N // NCORES          # 512 tokens per core
DH = D // H                # 256
EPS = 1e-5
KC = D // 128              # 8 feature chunks of 128

_CACHE = {}


# ----------------------------------------------------------------------------
# kernel body
# ----------------------------------------------------------------------------

def _body(nc, tc, io):
    P = 128

    # ---- persistent SBUF activation tensors (feature-major [128, TOK]) ----
    xA = [nc.alloc_sbuf_tensor(f"xA{i}", [P, TOK], FP32R).ap() for i in range(KC)]
    xB = [nc.alloc_sbuf_tensor(f"xB{i}", [P, TOK], FP32R).ap() for i in range(KC)]
    qT = [nc.alloc_sbuf_tensor(f"qT{i}", [P, TOK], FP32R).ap() for i in range(KC)]
    hT = [nc.alloc_sbuf_tensor(f"hT{i}", [P, TOK], FP32R).ap() for i in range(2 * KC)]
    qTb = [nc.alloc_sbuf_tensor(f"qTb{i}", [P, TOK], BF16).ap() for i in range(KC)]
    o_acc = [[nc.alloc_sbuf_tensor(f"oacc{h}_{m}", [P, DH + 2], FP32).ap()
              for m in range(4)] for h in range(H)]
    vs_acc = [nc.alloc_sbuf_tensor(f"vsacc{h}", [1, DH + 2], FP32).ap()
              for h in range(H)]

    with (
        tc.tile_pool(name="const", bufs=1) as cp,
        tc.tile_pool(name="wp", bufs=8) as wp,
        tc.tile_pool(name="sp", bufs=4) as sp,
        tc.tile_pool(name="dram", bufs=1, space="DRAM") as dp,
    ):
        # ---- constants ----
        ones_col = cp.tile([P, 1], FP32R, tag="ones_col")
        nc.gpsimd.dma_start(ones_col[:], io["c_ones"][0:128].rearrange("(p o) -> p o", o=1))
        ones_row = cp.tile([1, P], FP32R, tag="ones_row")
        nc.gpsimd.dma_start(ones_row[:], io["c_ones"][0:128].rearrange("(o p) -> o p", o=1))
        onesb_col = cp.tile([P, 1], BF16, tag="onesb_col")
        nc.gpsimd.dma_start(onesb_col[:], io["c_onesb"][0:128].rearrange("(p o) -> p o", o=1))
        onesb_col2 = cp.tile([P, 2], BF16, tag="onesb_col2")
        nc.gpsimd.dma_start(onesb_col2[:], io["c_onesb"][0:256].rearrange("(p o) -> p o", o=2))
        onesb_col8 = cp.tile([P, 8], BF16, tag="onesb_col8")
        nc.gpsimd.dma_start(onesb_col8[:], io["c_onesb"][0:1024].rearrange("(p o) -> p o", o=8))
        onesb_col4 = cp.tile([P, 4], BF16, tag="onesb_col4")
        nc.gpsimd.dma_start(onesb_col4[:], io["c_onesb"][0:512].rearrange("(p o) -> p o", o=4))
        onesb_row = cp.tile([1, P], BF16, tag="onesb_row")
        nc.gpsimd.dma_start(onesb_row[:], io["c_onesb"][0:128].rearrange("(o p) -> o p", o=1))
        eye = cp.tile([P, P], FP32R, tag="eye")
        nc.gpsimd.dma_start(eye[:], io["c_eye"][:, :])
        eps_t = cp.tile([1, 1], FP32, tag="eps")
        nc.vector.memset(eps_t[:], EPS)
        vb_row = cp.tile([1, D], FP32R, tag="vb_row")
        nc.gpsimd.dma_start(vb_row[:], io["vb"][:].rearrange("(o d) -> o d", o=1))

        def vec_tile(name, length):
            cols = length // P
            t = cp.tile([P, cols], FP32, tag=f"vec_{name}")
            nc.gpsimd.dma_start(t[:], io[name][:].rearrange("(c p) -> p c", p=P))
            return t

        qkb_t = vec_tile("qkb", 2 * D)
        ob_t = vec_tile("ob", D)
        f1b_t = vec_tile("f1b", DFF)
        f2b_t = vec_tile("f2b", D)
        ln1g_t = vec_tile("ln1g", D)
        ln1b_t = vec_tile("ln1b", D)
        ln2g_t = vec_tile("ln2g", D)
        ln2b_t = vec_tile("ln2b", D)
        ffb_t = vec_tile("ffb", D)
        cfb_t = vec_tile("cfb", D)
        k1b_t = vec_tile("k1b", D)
        k2b_t = vec_tile("k2b", D)
        outb_t = vec_tile("outb", D)
        moeb_t = [vec_tile(f"moeb{l}", D) for l in range(L)]

        # ---- DRAM buffers for the chunked bf16 K/V exchange ----
        kT_loc_j = [dp.tile([D, P], BF16, tag=f"kT_loc{j}", name=f"kT_loc{j}")
                    for j in range(4)]
        v_loc_j = [dp.tile([P, D], BF16, tag=f"v_loc{j}", name=f"v_loc{j}")
                   for j in range(4)]
        kT_all_j = [dp.tile([NCORES * D, P], BF16, tag=f"kT_all{j}",
                            name=f"kT_all{j}", addr_space="Shared")
                    for j in range(4)]
        v_all_j = [dp.tile([NCORES * P, D], BF16, tag=f"v_all{j}",
                           name=f"v_all{j}", addr_space="Shared")
                   for j in range(4)]

        # ------------------------------------------------------------------
        # dense feature-major GEMM:  out^T[M, TOK] = W[K, M]^T-contracted x^T
        # ------------------------------------------------------------------
        def gemm_fm(w_ap, K, M, x_tiles, out_tiles, bias_tile=None, bias_col0=0,
                    relu=False, out_dt=FP32R, psum_pool=None):
            kc = K // P
            for half in range(M // 1024):
                pss = [psum_pool.tile([P, TOK], FP32, tag="mm", bufs=8,
                                      name=f"psg{half}_{i}") for i in range(8)]
                for kk in range(kc // 2):
                    wt = wp.tile([P, 2048], FP32R, tag="w", bufs=3)
                    eng = nc.sync if kk % 2 == 0 else nc.scalar
                    eng.dma_start(
                        wt[:].rearrange("p (a c) -> p a c", a=2),
                        w_ap[kk * 256:(kk + 1) * 256,
                             half * 1024:(half + 1) * 1024].rearrange(
                                 "(a p) c -> p a c", p=P))
                    for k2 in range(2):
                        k = kk * 2 + k2
                        for m2 in range(8):
                            nc.tensor.matmul(
                                pss[m2][:], wt[:, k2 * 1024 + m2 * P:
                                               k2 * 1024 + (m2 + 1) * P],
                                x_tiles[k][:],
                                start=(k == 0), stop=(k == kc - 1))
                for m2 in range(8):
                    m = half * 8 + m2
                    if bias_tile is not None:
                        b = bias_tile[:, bias_col0 + m:bias_col0 + m + 1]
                        func = AF.Relu if relu else AF.Identity
                    else:
                        b = 0.0
                        func = AF.Relu if relu else AF.Copy
                    nc.scalar.activation(out_tiles[m][:], pss[m2][:], func, bias=b)

        # ------------------------------------------------------------------
        # layernorm over features (feature-major tiles)
        # ------------------------------------------------------------------
        def layernorm(in_tiles, out_tiles, g_t, b_t, psum_pool, idx):
            # partition-dim sums via ones-matmuls
            mu_ps = psum_pool.tile([P, TOK], FP32, tag="mm", bufs=8)
            sq_ps = psum_pool.tile([P, TOK], FP32, tag="mm", bufs=8)
            sqs = []
            for k in range(KC):
                sq = sp.tile([P, TOK], FP32R, tag="ev", bufs=3, name=f"lnsq{idx}_{k}")
                nc.vector.tensor_mul(sq[:], in_tiles[k][:], in_tiles[k][:])
                sqs.append(sq)
            for k in range(KC):
                nc.tensor.matmul(mu_ps[0:1, :], ones_col[:], in_tiles[k][:],
                                 start=(k == 0), stop=(k == KC - 1))
                nc.tensor.matmul(sq_ps[0:1, :], ones_col[:], sqs[k][:],
                                 start=(k == 0), stop=(k == KC - 1))
            mu_row = sp.tile([1, TOK], FP32R, tag="row_r", bufs=2, name=f"lnmu{idx}")
            nc.scalar.activation(mu_row[:], mu_ps[0:1, :], AF.Copy, scale=1.0 / D)
            m2_row = sp.tile([1, TOK], FP32, tag="row", bufs=3, name=f"lnm2{idx}")
            nc.scalar.activation(m2_row[:], sq_ps[0:1, :], AF.Copy, scale=1.0 / D)
            var_row = sp.tile([1, TOK], FP32, tag="row", bufs=3, name=f"lnvar{idx}")
            # var = E[x^2] - mu^2  (mu in fp32r costs ~1e-4 rel on mu only)
            musq = sp.tile([1, TOK], FP32, tag="row", bufs=3, name=f"lnmusq{idx}")
            nc.vector.tensor_mul(musq[:], mu_row[:], mu_row[:])
            nc.vector.tensor_sub(var_row[:], m2_row[:], musq[:])
            std_row = sp.tile([1, TOK], FP32, tag="row", bufs=3, name=f"lnstd{idx}")
            nc.scalar.activation(std_row[:], var_row[:], AF.Sqrt, bias=eps_t[:])
            rstd_row = sp.tile([1, TOK], FP32R, tag="row_r", bufs=2, name=f"lnrstd{idx}")
            nc.vector.reciprocal(rstd_row[:], std_row[:])
            # broadcast mu & rstd across partitions via K=1 matmuls
            mu_bps = psum_pool.tile([P, TOK], FP32, tag="mm", bufs=8)
            nc.tensor.matmul(mu_bps[:], ones_row[:], mu_row[:], start=True, stop=True)
            mu_b = sp.tile([P, TOK], FP32, tag="lnb", bufs=2, name=f"lnmub{idx}")
            nc.vector.tensor_copy(mu_b[:], mu_bps[:])
            rs_bps = psum_pool.tile([P, TOK], FP32, tag="mm", bufs=8)
            nc.tensor.matmul(rs_bps[:], ones_row[:], rstd_row[:], start=True, stop=True)
            rs_b = sp.tile([P, TOK], FP32, tag="lnb", bufs=2, name=f"lnrsb{idx}")
            nc.vector.tensor_copy(rs_b[:], rs_bps[:])
            for k in range(KC):
                t1 = sp.tile([P, TOK], FP32, tag="ev", bufs=3, name=f"lnt1_{idx}_{k}")
                nc.vector.tensor_sub(t1[:], in_tiles[k][:], mu_b[:])
                t2 = sp.tile([P, TOK], FP32, tag="ev", bufs=3, name=f"lnt2_{idx}_{k}")
                nc.vector.tensor_mul(t2[:], t1[:], rs_b[:])
                nc.scalar.activation(out_tiles[k][:], t2[:], AF.Identity,
                                     scale=g_t[:, k:k + 1], bias=b_t[:, k:k + 1])

        # ==================================================================
        # phase 1: input + MoE layers (3 dense GEMMs with averaged experts)
        # ==================================================================
        with tc.tile_pool(name="pg", bufs=6, space="PSUM") as pg:
            for i in range(KC):
                nc.sync.dma_start(xA[i][:], io["xT"][i * P:(i + 1) * P, :])
            cur, nxt = xA, xB
            for l in range(L):
                gemm_fm(io["moew"][l], D, D, cur, nxt,
                        bias_tile=moeb_t[l], psum_pool=pg)
                cur, nxt = nxt, cur
            # after L=3 layers: cur == xB holds post-MoE x^T
            x3 = cur
            assert x3 is xB

            # ==============================================================
            # phase 2: k^T first (feeds AllGather ASAP), then v, then q
            # ==============================================================
            pss = [pg.tile([P, TOK], FP32, tag="mm", bufs=8,
                           name=f"psk_{i}") for i in range(8)]
            for kk in range(KC // 2):
                wt = wp.tile([P, 2048], FP32R, tag="w", bufs=3)
                (nc.sync if kk % 2 == 0 else nc.scalar).dma_start(
                    wt[:].rearrange("p (a c) -> p a c", a=2),
                    io["qkw"][kk * 256:(kk + 1) * 256, 1024:2048].rearrange("(a p) c -> p a c", p=P))
                for k2 in range(2):
                    k = kk * 2 + k2
                    for m2 in range(8):
                        nc.tensor.matmul(
                            pss[m2][:], wt[:, k2 * 1024 + m2 * P:
                                           k2 * 1024 + (m2 + 1) * P],
                            x3[k][:], start=(k == 0), stop=(k == KC - 1))
            for m2 in range(8):
                kt_ev = sp.tile([P, TOK], BF16, tag="evb", bufs=2, name=f"ktev{m2}")
                nc.scalar.activation(kt_ev[:], pss[m2][:], AF.Identity,
                                     bias=qkb_t[:, 8 + m2:9 + m2])
                for j in range(4):
                    nc.sync.dma_start(
                        kT_loc_j[j][m2 * P:(m2 + 1) * P, :],
                        kt_ev[:, j * P:(j + 1) * P])

            # v token-major (bf16): out[tok, feat]; x^T slices as stationary
            pss = [pg.tile([P, TOK], FP32, tag="mm", bufs=8,
                           name=f"psv_{i}") for i in range(8)]
            for kk in range(KC // 2):
                wt = wp.tile([P, 2048], FP32R, tag="w", bufs=3)
                (nc.sync if kk % 2 == 0 else nc.scalar).dma_start(
                    wt[:].rearrange("p (a c) -> p a c", a=2),
                    io["vw"][kk * 256:(kk + 1) * 256, :].rearrange(
                        "(a p) c -> p a c", p=P))
                for k2 in range(2):
                    k = kk * 2 + k2
                    for mt in range(4):
                        for n in range(2):
                            nc.tensor.matmul(
                                pss[mt * 2 + n][:], x3[k][:, mt * P:(mt + 1) * P],
                                wt[:, k2 * 1024 + n * 512:k2 * 1024 + (n + 1) * 512],
                                start=(k == 0), stop=False)
            for mt in range(4):
                for n in range(2):
                    nc.tensor.matmul(pss[mt * 2 + n][:], ones_row[:],
                                     vb_row[0:1, n * 512:(n + 1) * 512],
                                     start=False, stop=True)
                    v_ev = sp.tile([P, TOK], BF16, tag="evb", bufs=2, name=f"vev{n}_{mt}")
                    nc.vector.tensor_copy(v_ev[:], pss[mt * 2 + n][:])
                    nc.sync.dma_start(
                        v_loc_j[mt][:, n * 512:(n + 1) * 512], v_ev[:])

            # chunked AllGathers, interleaved so attention can stream chunk 0 asap
            for j in range(4):
                nc.gpsimd.collective_compute(
                    "AllGather", ALU.bypass,
                    replica_groups=[list(range(NCORES))],
                    ins=[kT_loc_j[j].opt()], outs=[kT_all_j[j].opt()])
                nc.gpsimd.collective_compute(
                    "AllGather", ALU.bypass,
                    replica_groups=[list(range(NCORES))],
                    ins=[v_loc_j[j].opt()], outs=[v_all_j[j].opt()])

            # q^T (bf16) into qTb
            pss = [pg.tile([P, TOK], FP32, tag="mm", bufs=8,
                           name=f"psq_{i}") for i in range(8)]
            for kk in range(KC // 2):
                wt = wp.tile([P, 2048], FP32R, tag="w", bufs=3)
                (nc.sync if kk % 2 == 0 else nc.scalar).dma_start(
                    wt[:].rearrange("p (a c) -> p a c", a=2),
                    io["qkw"][kk * 256:(kk + 1) * 256, 0:1024].rearrange("(a p) c -> p a c", p=P))
                for k2 in range(2):
                    k = kk * 2 + k2
                    for m2 in range(8):
                        nc.tensor.matmul(
                            pss[m2][:], wt[:, k2 * 1024 + m2 * P:
                                           k2 * 1024 + (m2 + 1) * P],
                            x3[k][:], start=(k == 0), stop=(k == KC - 1))
            for m2 in range(8):
                nc.scalar.activation(qTb[m2][:], pss[m2][:], AF.Identity,
                                     bias=qkb_t[:, m2:m2 + 1])

        # ==================================================================
        # phase 3: attention, chunk-major streaming over the AllGathered K/V
        #   exp(S) = 1 + em1;  O = (sum_t V + sum_t em1*V) / (4096 + sum_t em1)
        #   per-chunk partial O accumulates in SBUF so chunk demand is even.
        # ==================================================================
        oT = xA  # feature-major attention output accumulates into xA slots
        with (
            tc.tile_pool(name="po", bufs=1, space="PSUM") as po,
            tc.tile_pool(name="ps_s", bufs=2, space="PSUM") as ps_s,
            tc.tile_pool(name="ps_t", bufs=1, space="PSUM") as ps_t,
        ):
            for j in range(4):
                ksrc = kT_all_j[j].rearrange("(r q p) c -> p r q c", r=NCORES, q=8)
                vsrc = v_all_j[j].rearrange("(r p) c -> p r c", r=NCORES)
                ktf = []
                vpf = []
                for r in range(NCORES):
                    kt = sp.tile([P, 1024], BF16, tag="ktf", bufs=8,
                                 name=f"ktf{j}_{r}")
                    nc.gpsimd.dma_start(kt[:].rearrange("p (q c) -> p q c", q=8),
                                        ksrc[:, r, :, :])
                    ktf.append(kt)
                    vp = sp.tile([P, 4 * (DH + 2)], BF16, tag="vpf", bufs=8,
                                 name=f"vpf{j}_{r}")
                    vpr = vp[:].rearrange("p (g x) -> p g x", g=4)
                    nc.gpsimd.dma_start(
                        vpr[:, :, 0:DH],
                        vsrc[:, r, :].rearrange("p (g c) -> p g c", g=4))
                    nc.vector.tensor_copy(
                        vpr[:, :, DH:DH + 2],
                        onesb_col8[:].rearrange("p (g x) -> p g x", g=4))
                    vpf.append(vp)
                for h in range(H):
                    o_ps = [po.tile([P, DH + 2], FP32, tag=f"o{m}",
                                    name=f"ops{j}_{h}_{m}") for m in range(4)]
                    vs_ps = po.tile([1, DH + 2], FP32, tag="vs", name=f"vsps{j}_{h}")
                    for r in range(NCORES):
                        vps = vpf[r][:, h * (DH + 2):(h + 1) * (DH + 2)]
                        st = ps_s.tile([P, TOK], FP32, tag="st")
                        nc.tensor.matmul(st[:],
                                         ktf[r][:, (2 * h) * P:(2 * h + 1) * P],
                                         qTb[2 * h][:], start=True, stop=False)
                        nc.tensor.matmul(st[:],
                                         ktf[r][:, (2 * h + 1) * P:(2 * h + 2) * P],
                                         qTb[2 * h + 1][:],
                                         start=False, stop=True)
                        esf = sp.tile([P, TOK], FP32, tag="esf", bufs=2,
                                      name=f"esf{h}_{j}_{r}")
                        nc.scalar.activation(esf[:], st[:], AF.Exp,
                                             scale=1.0 / 16.0)
                        es = sp.tile([P, TOK], BF16, tag="es", bufs=2,
                                     name=f"es{h}_{j}_{r}")
                        nc.vector.tensor_scalar_add(es[:], esf[:], -1.0)
                        first = (r == 0)
                        last = (r == NCORES - 1)
                        nc.tensor.matmul(vs_ps[:], onesb_col[:], vps,
                                         start=first, stop=last,
                                         skip_group_check=True)
                        for m in range(4):
                            nc.tensor.matmul(
                                o_ps[m][:], es[:, m * P:(m + 1) * P], vps,
                                start=first, stop=last,
                                skip_group_check=True)
                    # fold this chunk's partials into the SBUF accumulators
                    if j == 0:
                        nc.vector.tensor_copy(vs_acc[h][:], vs_ps[:])
                        for m in range(4):
                            nc.vector.tensor_copy(o_acc[h][m][:], o_ps[m][:])
                    else:
                        nc.vector.tensor_add(vs_acc[h][:], vs_acc[h][:], vs_ps[:])
                        for m in range(4):
                            nc.vector.tensor_add(o_acc[h][m][:], o_acc[h][m][:],
                                                 o_ps[m][:])
            # epilogue: add uniform part, normalize, transpose to feature-major
            for h in range(H):
                vsum_sb = sp.tile([1, DH + 2], BF16, tag="vsum", bufs=1, name=f"vsum{h}")
                nc.vector.tensor_copy(vsum_sb[:], vs_acc[h][:])
                for m in range(4):
                    bc_ps = ps_s.tile([P, DH + 2], FP32, tag="st",
                                      name=f"bc{h}_{m}")
                    nc.tensor.matmul(bc_ps[:], onesb_row[:], vsum_sb[:],
                                     start=True, stop=True, skip_group_check=True)
                    of = sp.tile([P, DH + 2], FP32, tag="of", bufs=2, name=f"of{h}_{m}")
                    nc.vector.tensor_add(of[:], o_acc[h][m][:], bc_ps[:])
                    recip = sp.tile([P, 1], FP32, tag="rc", bufs=2, name=f"rc{h}_{m}")
                    nc.vector.reciprocal(recip[:], of[:, DH:DH + 1])
                    osc = sp.tile([P, DH], FP32R, tag="osc", bufs=2, name=f"osc{h}_{m}")
                    nc.vector.tensor_scalar_mul(osc[:], of[:, 0:DH], recip[:])
                    for d2 in range(2):
                        tp = ps_t.tile([P, P], FP32R, tag="tp")
                        nc.tensor.transpose(tp[:], osc[:, d2 * P:(d2 + 1) * P], eye[:])
                        nc.vector.tensor_copy(
                            oT[2 * h + d2][:, m * P:(m + 1) * P], tp[:])

        # ==================================================================
        # phase 4: o-proj + LN1 + FFN + LN2 + trailing dense stack
        # ==================================================================
        with tc.tile_pool(name="pg2", bufs=6, space="PSUM") as pg2:
            gemm_fm(io["ow"], D, D, oT, qT, bias_tile=ob_t, psum_pool=pg2)
            for i in range(KC):
                nc.vector.tensor_add(xB[i][:], xB[i][:], qT[i][:])
            y1 = [None] * KC
            for i in range(KC):
                y1[i] = xA[i]
            layernorm(xB, y1, ln1g_t, ln1b_t, pg2, 0)
            gemm_fm(io["f1w"], D, DFF, y1, hT, bias_tile=f1b_t, relu=True,
                    psum_pool=pg2)
            gemm_fm(io["f2w"], DFF, D, hT, qT, bias_tile=f2b_t, psum_pool=pg2)
            for i in range(KC):
                nc.vector.tensor_add(xB[i][:], y1[i][:], qT[i][:])
            y2 = xA  # y1 dead after the add above
            layernorm(xB, y2, ln2g_t, ln2b_t, pg2, 1)
            gemm_fm(io["ffw"], D, D, y2, qT, bias_tile=ffb_t, psum_pool=pg2)
            gemm_fm(io["cfw"], D, D, qT, xB, bias_tile=cfb_t, psum_pool=pg2)
            gemm_fm(io["k1w"], D, D, xB, xA, bias_tile=k1b_t, relu=True,
                    psum_pool=pg2)
            gemm_fm(io["k2w"], D, D, xA, qT, bias_tile=k2b_t, psum_pool=pg2)
            # final GEMM: evict fp32 and DMA out
            pss = [pg2.tile([P, TOK], FP32, tag="mm", bufs=8,
                            name=f"psout_{i}") for i in range(8)]
            for kk in range(KC // 2):
                wt = wp.tile([P, 2048], FP32R, tag="w", bufs=3)
                (nc.sync if kk % 2 == 0 else nc.scalar).dma_start(
                    wt[:].rearrange("p (a c) -> p a c", a=2),
                    io["outw"][kk * 256:(kk + 1) * 256, :].rearrange(
                        "(a p) c -> p a c", p=P))
                for k2 in range(2):
                    k = kk * 2 + k2
                    for m2 in range(8):
                        nc.tensor.matmul(
                            pss[m2][:], wt[:, k2 * 1024 + m2 * P:
                                           k2 * 1024 + (m2 + 1) * P],
                            qT[k][:], start=(k == 0), stop=(k == KC - 1))
            for m2 in range(8):
                fin = sp.tile([P, TOK], FP32, tag="ev", bufs=3, name=f"fin{m2}")
                nc.scalar.activation(fin[:], pss[m2][:], AF.Identity,
                                     bias=outb_t[:, m2:m2 + 1])
                nc.sync.dma_start(io["outT"][m2 * P:(m2 + 1) * P, :], fin[:])


def _build():
    nc = bacc.Bacc("TRN2", debug=False, num_devices=NCORES)

    def din(name, shape, dt=FP32R):
        return nc.dram_tensor(name, shape, dt, kind="ExternalInput").ap()

    io = {
        "xT": din("xT", [D, TOK]),
        "moew": din("moew", [L, D, D]),
        "qkw": din("qkw", [D, 2 * D]),
        "vw": din("vw", [D, D]),
        "vb": din("vb", [D]),
        "ow": din("ow", [D, D]),
        "f1w": din("f1w", [D, DFF]),
        "f2w": din("f2w", [DFF, D]),
        "ffw": din("ffw", [D, D]),
        "cfw": din("cfw", [D, D]),
        "k1w": din("k1w", [D, D]),
        "k2w": din("k2w", [D, D]),
        "outw": din("outw", [D, D]),
        "c_ones": din("c_ones", [256]),
        "c_onesb": din("c_onesb", [1024], BF16),
        "c_eye": din("c_eye", [128, 128]),
    }
    for name, shape in [("qkb", [2 * D]), ("ob", [D]), ("f1b", [DFF]),
                        ("f2b", [D]), ("ln1g", [D]), ("ln1b", [D]),
                        ("ln2g", [D]), ("ln2b", [D]), ("ffb", [D]),
                        ("cfb", [D]), ("k1b", [D]), ("k2b", [D]),
                        ("outb", [D])]:
        io[name] = din(name, shape, FP32)
    for l in range(L):
        io[f"moeb{l}"] = din(f"moeb{l}", [D], FP32)
    io["outT"] = nc.dram_tensor("outT", [D, TOK], FP32, kind="ExternalOutput").ap()

    with nc.allow_low_precision("fp32r matmul pipeline"):
        with tile.TileContext(nc) as tc:
            _body(nc, tc, io)
    nc.compile()
    return nc


# ----------------------------------------------------------------------------
# host side
# ----------------------------------------------------------------------------

def _route(x, gw, gb, ew, eb):
    """Replicates the degenerate routing: top-2 experts of token 0, averaged."""
    x0 = x[0].astype(np.float64)
    Ws, bs = [], []
    for l in range(L):
        s = x0 @ gw[l].astype(np.float64) + gb[l].astype(np.float64)
        sel = np.argsort(-s, kind="stable")[:2]
        W = (ew[l][sel[0]].astype(np.float64) + ew[l][sel[1]].astype(np.float64)) * 0.5
        b = (eb[l][sel[0]].astype(np.float64) + eb[l][sel[1]].astype(np.float64)) * 0.5
        Ws.append(W.astype(np.float32))
        bs.append(b.astype(np.float32))
        x0 = x0 @ W + b
    return Ws, bs


def kernel(x, gw, gb, ew, eb, qkvw, qkvb, ow, ob, ln1g, ln1b, ln2g, ln2b,
           f1w, f1b, f2w, f2b, ffw, ffb, cfw, cfb, k1w, k1b, k2w, k2b,
           outw, outb):
    x = np.asarray(x, dtype=np.float32)
    gw, gb = np.asarray(gw, np.float32), np.asarray(gb, np.float32)
    ew, eb = np.asarray(ew, np.float32), np.asarray(eb, np.float32)
    qkvw, qkvb = np.asarray(qkvw, np.float32), np.asarray(qkvb, np.float32)

    Ws, bs = _route(x, gw, gb, ew, eb)
    moew = np.ascontiguousarray(np.stack(Ws))              # [L, D, D]

    if "nc" not in _CACHE:
        _CACHE["nc"] = _build()
    nc = _CACHE["nc"]

    shared = {
        "moew": moew,
        "qkw": np.ascontiguousarray(qkvw[:, :2 * D]),
        "vw": np.ascontiguousarray(qkvw[:, 2 * D:]),
        "vb": np.ascontiguousarray(qkvb[2 * D:]),
        "qkb": np.ascontiguousarray(qkvb[:2 * D]),
        "ow": np.asarray(ow, np.float32), "ob": np.asarray(ob, np.float32),
        "f1w": np.asarray(f1w, np.float32), "f1b": np.asarray(f1b, np.float32),
        "f2w": np.asarray(f2w, np.float32), "f2b": np.asarray(f2b, np.float32),
        "ln1g": np.asarray(ln1g, np.float32), "ln1b": np.asarray(ln1b, np.float32),
        "ln2g": np.asarray(ln2g, np.float32), "ln2b": np.asarray(ln2b, np.float32),
        "ffw": np.asarray(ffw, np.float32), "ffb": np.asarray(ffb, np.float32),
        "cfw": np.asarray(cfw, np.float32), "cfb": np.asarray(cfb, np.float32),
        "k1w": np.asarray(k1w, np.float32), "k1b": np.asarray(k1b, np.float32),
        "k2w": np.asarray(k2w, np.float32), "k2b": np.asarray(k2b, np.float32),
        "outw": np.asarray(outw, np.float32), "outb": np.asarray(outb, np.float32),
        "c_ones": np.ones(256, np.float32),
        "c_onesb": np.ones(1024, ml_dtypes.bfloat16),
        "c_eye": np.eye(128, dtype=np.float32),
    }
    for l in range(L):
        shared[f"moeb{l}"] = bs[l]

    in_maps = []
    for c in range(NCORES):
        m = dict(shared)
        m["xT"] = np.ascontiguousarray(x[c * TOK:(c + 1) * TOK].T)
        in_maps.append(m)

    _CACHE["in_maps"] = in_maps
    res = bass_utils.run_bass_kernel_spmd(nc, in_maps, core_ids=list(range(NCORES)))
    _CACHE["last_result"] = res

    out = np.empty((N, D), np.float32)
    for c in range(NCORES):
        out[c * TOK:(c + 1) * TOK, :] = res.results[c]["outT"].T
    return out



# revision 12
# speedup vs baseline: 1.4752x; 1.4752x over previous
"""Trainium2 Bass kernel for nn_LiquidModel (moe_routing) — v2.

Key structure (host does all algebraic folding; device does 12 GEMM-units):
 - Degenerate MoE routing: experts chosen by token 0, averaged -> each MoE
   layer is one dense GEMM; all three layers have NO nonlinearity between
   them, so host folds W1@W2@W3 (f64) into ONE GEMM.  Same for ffw@cfw and
   k2w@outw in the trailing stack.
 - All dense GEMMs run bf16 (weights converted on host, activations evicted
   to bf16; fp32 PSUM accumulate).  Residual stream kept in fp32 SBUF.
 - Attention: K/V/Q quantized to fp8(e4m3, x16 scale) on device.  K+V are
   exchanged via 4 chunked AllGathers (fp8 halves the serialized collective
   time).  Scores and attn@V run as DoubleRow fp8 matmuls (2 contraction
   rows per partition).  The uniform part of softmax (exp(S) = 1 + es) is
   computed EXACTLY: sum_t v_t comes from a host-side column-sum identity
   (colsum(v) = (colsum(x) @ Wmoe + N*bmoe) @ vw + N*vb), so fp8 noise only
   touches the small es*v correction term.
 - LN stats via ones-matmuls (fp32r), normalize straight to bf16.
"""
import ml_dtypes
import numpy as np

import concourse.bacc as bacc
import concourse.bass as bass
import concourse.mybir as mybir
import concourse.tile as tile
from concourse import bass_utils

FP32 = mybir.dt.float32
FP32R = mybir.dt.float32r
BF16 = mybir.dt.bfloat16
FP8 = mybir.dt.float8e4
AF = mybir.ActivationFunctionType
ALU = mybir.AluOpType
DR = mybir.MatmulPerfMode.DoubleRow

NCORES = 8
N, D, DFF, H, L = 4096, 1024, 2048, 4, 3
TOK = N // NCORES          # 512 tokens per core
DH = D // H                # 256
EPS = 1e-5
KC = D // 128              # 8 feature chunks of 128
P = 128
JC = 4                     # token quarters for the kv exchange
JT = TOK // JC             # 128 tokens per exchange chunk

_CACHE = {}


def _body(nc, tc, io):
    # ---- persistent SBUF activation tensors (feature-major [128, TOK]) ----
    xin = [nc.alloc_sbuf_tensor(f"xin{i}", [P, TOK], BF16).ap() for i in range(KC)]
    x3b = [nc.alloc_sbuf_tensor(f"x3b{i}", [P, TOK], BF16).ap() for i in range(KC)]
    xR = [nc.alloc_sbuf_tensor(f"xR{i}", [P, TOK], FP32R).ap() for i in range(KC)]
    yA = [nc.alloc_sbuf_tensor(f"yA{i}", [P, TOK], BF16).ap() for i in range(KC)]
    yB = [nc.alloc_sbuf_tensor(f"yB{i}", [P, TOK], BF16).ap() for i in range(KC)]
    hT = [nc.alloc_sbuf_tensor(f"hT{i}", [P, TOK], BF16).ap() for i in range(2 * KC)]
    q8 = [nc.alloc_sbuf_tensor(f"q8_{h}", [P, 2 * TOK], FP8).ap() for h in range(H)]
    oT = xin  # attention output reuses the input slots (dead after MoE)
    o_acc = [[nc.alloc_sbuf_tensor(f"oacc{h}_{m}", [P, DH + 2], FP32).ap()
              for m in range(4)] for h in range(H)]

    with (
        tc.tile_pool(name="const", bufs=1) as cp,
        tc.tile_pool(name="wp", bufs=6) as wp,
        tc.tile_pool(name="sp", bufs=4) as sp,
        tc.tile_pool(name="dram", bufs=1, space="DRAM") as dp,
    ):
        # ---- constants (gpsimd queue; issued before the sync AllGather) ----
        onesb_col = cp.tile([P, 1], BF16, tag="onesb_col")
        nc.gpsimd.dma_start(onesb_col[:], io["c_onesb"][0:128].rearrange("(p o) -> p o", o=1))
        onesb_row = cp.tile([1, P], BF16, tag="onesb_row")
        nc.gpsimd.dma_start(onesb_row[:], io["c_onesb"][0:128].rearrange("(o p) -> o p", o=1))
        ones_col = cp.tile([P, 1], FP32R, tag="ones_col")
        nc.gpsimd.dma_start(ones_col[:], io["c_ones"][0:128].rearrange("(p o) -> p o", o=1))
        ones_row = cp.tile([1, P], FP32R, tag="ones_row")
        nc.gpsimd.dma_start(ones_row[:], io["c_ones"][0:128].rearrange("(o p) -> o p", o=1))
        eye = cp.tile([P, P], FP32R, tag="eye")
        nc.gpsimd.dma_start(eye[:], io["c_eye"][:, :])
        eps_t = cp.tile([1, 1], FP32, tag="eps")
        nc.vector.memset(eps_t[:], EPS)
        vb_row = cp.tile([1, D], BF16, tag="vb_row")
        nc.gpsimd.dma_start(vb_row[:], io["vbb"][:].rearrange("(o d) -> o d", o=1))
        vrow_r = cp.tile([1, H * (DH + 2)], FP32R, tag="vrow")
        nc.gpsimd.dma_start(vrow_r[:], io["vrow"][:].rearrange("(o d) -> o d", o=1))

        def vec_tile(name, length):
            cols = length // P
            t = cp.tile([P, cols], FP32, tag=f"vec_{name}")
            nc.gpsimd.dma_start(t[:], io[name][:].rearrange("(c p) -> p c", p=P))
            return t

        qkb16_t = vec_tile("qkb16", 2 * D)
        ob_t = vec_tile("ob", D)
        f1b_t = vec_tile("f1b", DFF)
        f2b_t = vec_tile("f2b", D)
        ln1g_t = vec_tile("ln1g", D)
        ln1b_t = vec_tile("ln1b", D)
        ln2g_t = vec_tile("ln2g", D)
        ln2b_t = vec_tile("ln2b", D)
        fcb_t = vec_tile("fcb", D)
        k1b_t = vec_tile("k1b", D)
        kob_t = vec_tile("kob", D)
        moeb_t = vec_tile("moeb", D)

        # ---- DRAM buffers for the chunked fp8 K/V exchange ----
        # kv_loc_j: rows 0..1023 = kT chunk [feat, JT tok]; rows 1024..2047
        # hold v chunk [JT tok, D feat] flattened as (t a) c -> t (a c).
        kv_loc = [dp.tile([2048, JT], FP8, tag=f"kv_loc{j}", name=f"kv_loc{j}")
                  for j in range(JC)]
        kv_all = [dp.tile([NCORES * 2048, JT], FP8, tag=f"kv_all{j}",
                          name=f"kv_all{j}", addr_space="Shared")
                  for j in range(JC)]
        sync_loc = dp.tile([8, 8], BF16, tag="sync_loc", name="sync_loc")
        sync_all = dp.tile([64, 8], BF16, tag="sync_all", name="sync_all",
                           addr_space="Shared")

        # ---- input x loads: chunks 0-3 on sync queue, 4-7 on gpsimd ----
        for i in range(KC):
            eng = nc.sync if i < 4 else nc.gpsimd
            eng.dma_start(xin[i][:], io["xT"][i * P:(i + 1) * P, :])

        # early barrier collective: absorbs first-use overhead + core drift
        nc.gpsimd.dma_start(sync_loc[:], io["c_onesb"][0:64].rearrange("(a b) -> a b", b=8))
        nc.gpsimd.collective_compute(
            "AllGather", ALU.bypass, replica_groups=[list(range(NCORES))],
            ins=[sync_loc.opt()], outs=[sync_all.opt()])

        # ------------------------------------------------------------------
        # dense feature-major GEMM:  psum[M-chunk, TOK] = W[K, M]^T-contr x^T
        # evict(m, pss) is called per output chunk m (128 features).
        # ------------------------------------------------------------------
        def gemm_fm(w_ap, K, M, x_tiles, evict, psum_pool, col0=0):
            kc = K // P
            for half in range(M // 1024):
                pss = [psum_pool.tile([P, TOK], FP32, tag="mm", bufs=8,
                                      name=f"ps{half}_{i}") for i in range(8)]
                for kk in range(kc // 2):
                    wt = wp.tile([P, 2048], BF16, tag="w", bufs=6)
                    eng = nc.sync if kk % 2 == 0 else nc.scalar
                    eng.dma_start(
                        wt[:].rearrange("p (a c) -> p a c", a=2),
                        w_ap[kk * 256:(kk + 1) * 256,
                             col0 + half * 1024:col0 + (half + 1) * 1024].rearrange(
                                 "(a p) c -> p a c", p=P))
                    for k2 in range(2):
                        k = kk * 2 + k2
                        for m2 in range(8):
                            nc.tensor.matmul(
                                pss[m2][:], wt[:, k2 * 1024 + m2 * P:
                                               k2 * 1024 + (m2 + 1) * P],
                                x_tiles[k][:],
                                start=(k == 0), stop=(k == kc - 1))
                for m2 in range(8):
                    evict(half * 8 + m2, pss[m2])

        # ------------------------------------------------------------------
        # layernorm over features: in fp32r tiles -> out bf16 tiles
        # ------------------------------------------------------------------
        def layernorm(in_tiles, out_tiles, g_t, b_t, psum_pool, idx):
            mu_ps = psum_pool.tile([P, TOK], FP32, tag="mm", bufs=8)
            sq_ps = psum_pool.tile([P, TOK], FP32, tag="mm", bufs=8)
            sqs = []
            for k in range(KC):
                sq = sp.tile([P, TOK], FP32R, tag="ev", bufs=3, name=f"lnsq{idx}_{k}")
                nc.vector.tensor_mul(sq[:], in_tiles[k][:], in_tiles[k][:])
                sqs.append(sq)
            for k in range(KC):
                nc.tensor.matmul(mu_ps[0:1, :], ones_col[:], in_tiles[k][:],
                                 start=(k == 0), stop=(k == KC - 1))
                nc.tensor.matmul(sq_ps[0:1, :], ones_col[:], sqs[k][:],
                                 start=(k == 0), stop=(k == KC - 1))
            mu_row = sp.tile([1, TOK], FP32R, tag="row_r", bufs=2, name=f"lnmu{idx}")
            nc.scalar.activation(mu_row[:], mu_ps[0:1, :], AF.Copy, scale=1.0 / D)
            m2_row = sp.tile([1, TOK], FP32, tag="row", bufs=3, name=f"lnm2{idx}")
            nc.scalar.activation(m2_row[:], sq_ps[0:1, :], AF.Copy, scale=1.0 / D)
            var_row = sp.tile([1, TOK], FP32, tag="row", bufs=3, name=f"lnvar{idx}")
            musq = sp.tile([1, TOK], FP32, tag="row", bufs=3, name=f"lnmusq{idx}")
            nc.vector.tensor_mul(musq[:], mu_row[:], mu_row[:])
            nc.vector.tensor_sub(var_row[:], m2_row[:], musq[:])
            std_row = sp.tile([1, TOK], FP32, tag="row", bufs=3, name=f"lnstd{idx}")
            nc.scalar.activation(std_row[:], var_row[:], AF.Sqrt, bias=eps_t[:])
            rstd_row = sp.tile([1, TOK], FP32R, tag="row_r", bufs=2, name=f"lnrstd{idx}")
            nc.vector.reciprocal(rstd_row[:], std_row[:])
            mu_bps = psum_pool.tile([P, TOK], FP32, tag="mm", bufs=8)
            nc.tensor.matmul(mu_bps[:], ones_row[:], mu_row[:], start=True, stop=True)
            mu_b = sp.tile([P, TOK], FP32, tag="lnb", bufs=2, name=f"lnmub{idx}")
            nc.vector.tensor_copy(mu_b[:], mu_bps[:])
            rs_bps = psum_pool.tile([P, TOK], FP32, tag="mm", bufs=8)
            nc.tensor.matmul(rs_bps[:], ones_row[:], rstd_row[:], start=True, stop=True)
            rs_b = sp.tile([P, TOK], FP32, tag="lnb", bufs=2, name=f"lnrsb{idx}")
            nc.vector.tensor_copy(rs_b[:], rs_bps[:])
            for k in range(KC):
                t1 = sp.tile([P, TOK], FP32, tag="ev", bufs=3, name=f"lnt1_{idx}_{k}")
                nc.vector.tensor_sub(t1[:], in_tiles[k][:], mu_b[:])
                t2 = sp.tile([P, TOK], FP32, tag="ev", bufs=3, name=f"lnt2_{idx}_{k}")
                nc.vector.tensor_mul(t2[:], t1[:], rs_b[:])
                nc.scalar.activation(out_tiles[k][:], t2[:], AF.Identity,
                                     scale=g_t[:, k:k + 1], bias=b_t[:, k:k + 1])

        # ==================================================================
        # phase 1: fused MoE (ONE dense GEMM)
        # ==================================================================
        with tc.tile_pool(name="pg", bufs=6, space="PSUM") as pg:
            def moe_evict(m, pss):
                nc.scalar.activation(x3b[m][:], pss[:], AF.Identity,
                                     bias=moeb_t[:, m:m + 1])
                nc.vector.tensor_scalar_add(xR[m][:], pss[:], moeb_t[:, m:m + 1])
            gemm_fm(io["moew"], D, D, xin, moe_evict, pg)

            # ==============================================================
            # phase 2: k first (feeds AllGather asap), then v, then q
            # ==============================================================
            def k_evict(m, pss):
                k8 = sp.tile([P, TOK], FP8, tag="ev8", bufs=2, name=f"k8_{m}")
                nc.scalar.activation(k8[:], pss[:], AF.Identity, scale=16.0,
                                     bias=qkb16_t[:, 8 + m:9 + m])
                for j in range(JC):
                    nc.sync.dma_start(kv_loc[j][m * P:(m + 1) * P, :],
                                      k8[:, j * JT:(j + 1) * JT])
            gemm_fm(io["qkw"], D, D, x3b, k_evict, pg, col0=1024)

            # v token-major: out[tok, feat]; x3 slices stationary, vw moving
            pss = [pg.tile([P, TOK], FP32, tag="mm", bufs=8,
                           name=f"psv_{i}") for i in range(8)]
            for kk in range(KC // 2):
                wt = wp.tile([P, 2048], BF16, tag="w", bufs=6)
                (nc.sync if kk % 2 == 0 else nc.scalar).dma_start(
                    wt[:].rearrange("p (a c) -> p a c", a=2),
                    io["vw"][kk * 256:(kk + 1) * 256, :].rearrange(
                        "(a p) c -> p a c", p=P))
                for k2 in range(2):
                    k = kk * 2 + k2
                    for mt in range(4):
                        for n in range(2):
                            nc.tensor.matmul(
                                pss[mt * 2 + n][:], x3b[k][:, mt * P:(mt + 1) * P],
                                wt[:, k2 * 1024 + n * 512:k2 * 1024 + (n + 1) * 512],
                                start=(k == 0), stop=False)
            for mt in range(4):
                vsec = kv_loc[mt][1024:2048, :].rearrange("(t a) c -> t (a c)", a=8)
                for n in range(2):
                    nc.tensor.matmul(pss[mt * 2 + n][:], onesb_row[:],
                                     vb_row[0:1, n * 512:(n + 1) * 512],
                                     start=False, stop=True)
                    v8 = sp.tile([P, TOK], FP8, tag="ev8", bufs=2, name=f"v8_{mt}_{n}")
                    nc.scalar.activation(v8[:], pss[mt * 2 + n][:], AF.Identity,
                                         scale=16.0)
                    nc.sync.dma_start(vsec[:, n * 512:(n + 1) * 512], v8[:])

            # q^T in fp8 (scaled x16), packed per head [P, 2*TOK]
            def q_evict(m, pss):
                h, s = m // 2, m % 2
                nc.scalar.activation(q8[h][:, s * TOK:(s + 1) * TOK], pss[:],
                                     AF.Identity, scale=16.0,
                                     bias=qkb16_t[:, m:m + 1])
            gemm_fm(io["qkw"], D, D, x3b, q_evict, pg, col0=0)

        # ==================================================================
        # phase 3: attention over AllGathered fp8 K/V, DoubleRow matmuls
        #   exp(S) = 1 + es;  uniform part Sum_t v is exact (host vrow).
        #   st = sum 256q*256k = 4096*S;  es8 = 128*es (fp8)
        # ==================================================================
        with (
            tc.tile_pool(name="po", bufs=1, space="PSUM") as po,
            tc.tile_pool(name="ps_s", bufs=3, space="PSUM") as ps_s,
            tc.tile_pool(name="ps_t", bufs=1, space="PSUM") as ps_t,
        ):
            for j in range(JC):
                nc.gpsimd.collective_compute(
                    "AllGather", ALU.bypass,
                    replica_groups=[list(range(NCORES))],
                    ins=[kv_loc[j].opt()], outs=[kv_all[j].opt()])
                # loads (gpsimd queue: naturally ordered after the AG)
                ktf = []
                for r in range(NCORES):
                    kt = sp.tile([P, 1024], FP8, tag="ktf", bufs=9,
                                 name=f"ktf{j}_{r}")
                    nc.gpsimd.dma_start(
                        kt[:].rearrange("p (g c) -> p g c", g=8),
                        kv_all[j][r * 2048:r * 2048 + 1024, :].rearrange(
                            "(g p) c -> p g c", p=P))
                    ktf.append(kt)
                vpf = []
                for pr in range(4):  # core pairs (2pr, 2pr+1)
                    vp = sp.tile([P, 2 * 4 * 260], FP8, tag="vpf",
                                 bufs=5, name=f"vpf{j}_{pr}")
                    # layout [p, s(2), h(4), x(260)]: 256 v cols + ones + pad
                    vp3 = vp[:].rearrange("p (s h x) -> p s h x", s=2, h=4)
                    for rr in range(2):
                        r = 2 * pr + rr
                        nc.gpsimd.dma_start(
                            vp3[:, rr, :, 0:256],
                            kv_all[j][r * 2048 + 1024:(r + 1) * 2048, :].rearrange(
                                "(t h b) c -> t h (b c)", h=4, b=2))
                    nc.vector.memset(vp3[:, :, :, 256:260], 1.0)
                    vpf.append(vp)
                # ---- scores + es for all (r, h) ----
                es8s = [[None] * H for _ in range(4)]
                for r in range(NCORES):
                    ktr = ktf[r][:].rearrange("p (g c) -> p g c", g=8)
                    for h in range(H):
                        st = ps_s.tile([P, TOK], FP32, tag="st")
                        nc.tensor.matmul(
                            st[:], ktr[:, 2 * h:2 * h + 2, :],
                            q8[h][:].rearrange("p (s c) -> p s c", s=2),
                            start=True, stop=True, perf_mode=DR)
                        pr, rr = r // 2, r % 2
                        if es8s[pr][h] is None:
                            es8s[pr][h] = sp.tile([P, 2 * TOK], FP8, tag="es8",
                                                  bufs=18, name=f"es8_{j}_{pr}_{h}")
                        dst = es8s[pr][h][:, rr * TOK:(rr + 1) * TOK]
                        if (4 * r + h) % 4 != 3:
                            esf = sp.tile([P, TOK], FP32, tag="esf", bufs=3,
                                          name=f"esf{j}_{r}_{h}")
                            nc.scalar.activation(esf[:], st[:], AF.Exp,
                                                 scale=1.0 / 4096.0)
                            nc.vector.tensor_scalar(dst, esf[:], 128.0, 128.0,
                                                    ALU.mult, ALU.subtract)
                        else:
                            w = sp.tile([P, TOK], FP32, tag="esw", bufs=3,
                                        name=f"esw{j}_{r}_{h}")
                            nc.vector.tensor_scalar(w[:], st[:], 2.0 ** -18,
                                                    2.0 ** -5, ALU.mult, ALU.add)
                            nc.vector.tensor_mul(dst, w[:], st[:])
                # ---- attn @ V, accumulate over pairs in PSUM per head ----
                for h in range(H):
                    o_ps = [po.tile([P, DH + 2], FP32, tag=f"o{m}", bufs=1,
                                    name=f"ops{j}_{h}_{m}") for m in range(4)]
                    for pr in range(4):
                        es3 = es8s[pr][h][:].rearrange("p (s c) -> p s c", s=2)
                        vp3 = vpf[pr][:].rearrange("p (s h x) -> p s h x",
                                                   s=2, h=4)
                        for m in range(4):
                            nc.tensor.matmul(
                                o_ps[m][:], es3[:, :, m * P:(m + 1) * P],
                                vp3[:, :, h, 0:DH + 2],
                                start=(pr == 0), stop=(pr == 3),
                                perf_mode=DR, skip_group_check=True)
                    for m in range(4):
                        if j == 0:
                            nc.vector.tensor_copy(o_acc[h][m][:], o_ps[m][:])
                        else:
                            nc.vector.tensor_add(o_acc[h][m][:], o_acc[h][m][:],
                                                 o_ps[m][:])
            # epilogue: add exact uniform part + denominator, normalize,
            # transpose to feature-major.  o_acc = [2048*sum(es v') | 128*sum es]
            # vrow = [2048*vsum_true | 128*4096];  result = 16*o_true,
            # the 1/16 is folded into ow on the host.
            for h in range(H):
                bc_ps = ps_s.tile([P, DH + 2], FP32, tag="st", name=f"bc{h}")
                nc.tensor.matmul(bc_ps[:], ones_row[:],
                                 vrow_r[0:1, h * (DH + 2):(h + 1) * (DH + 2)],
                                 start=True, stop=True, skip_group_check=True)
                for m in range(4):
                    of = sp.tile([P, DH + 2], FP32, tag="of", bufs=2, name=f"of{h}_{m}")
                    nc.vector.tensor_add(of[:], o_acc[h][m][:], bc_ps[:])
                    recip = sp.tile([P, 1], FP32, tag="rc", bufs=2, name=f"rc{h}_{m}")
                    nc.vector.reciprocal(recip[:], of[:, DH:DH + 1])
                    osc = sp.tile([P, DH], FP32R, tag="osc", bufs=2, name=f"osc{h}_{m}")
                    nc.vector.tensor_scalar_mul(osc[:], of[:, 0:DH], recip[:])
                    for d2 in range(2):
                        tp = ps_t.tile([P, P], FP32R, tag="tp")
                        nc.tensor.transpose(tp[:], osc[:, d2 * P:(d2 + 1) * P], eye[:])
                        nc.vector.tensor_copy(
                            oT[2 * h + d2][:, m * P:(m + 1) * P], tp[:])

        # ==================================================================
        # phase 4: o-proj + LN1 + FFN + LN2 + fused trailing stack
        # ==================================================================
        with tc.tile_pool(name="pg2", bufs=6, space="PSUM") as pg2:
            def oproj_evict(m, pss):
                t = sp.tile([P, TOK], BF16, tag="ev16", bufs=3, name=f"op{m}")
                nc.scalar.activation(t[:], pss[:], AF.Identity,
                                     bias=ob_t[:, m:m + 1])
                nc.vector.tensor_add(xR[m][:], xR[m][:], t[:])
            gemm_fm(io["ow"], D, D, oT, oproj_evict, pg2)
            layernorm(xR, yA, ln1g_t, ln1b_t, pg2, 0)

            def mk_evict(out_tiles, bias_t, relu=False):
                def ev(m, pss):
                    nc.scalar.activation(out_tiles[m][:], pss[:],
                                         AF.Relu if relu else AF.Identity,
                                         bias=bias_t[:, m:m + 1])
                return ev
            gemm_fm(io["f1w"], D, DFF, yA, mk_evict(hT, f1b_t, relu=True), pg2)

            def f2_evict(m, pss):
                t = sp.tile([P, TOK], BF16, tag="ev16", bufs=3, name=f"f2e{m}")
                nc.scalar.activation(t[:], pss[:], AF.Identity,
                                     bias=f2b_t[:, m:m + 1])
                nc.vector.tensor_add(xR[m][:], yA[m][:], t[:])
            gemm_fm(io["f2w"], DFF, D, hT, f2_evict, pg2)
            layernorm(xR, yB, ln2g_t, ln2b_t, pg2, 1)
            gemm_fm(io["fcw"], D, D, yB, mk_evict(yA, fcb_t), pg2)
            gemm_fm(io["k1w"], D, D, yA, mk_evict(yB, k1b_t, relu=True), pg2)

            def out_evict(m, pss):
                fin = sp.tile([P, TOK], FP32, tag="ev", bufs=3, name=f"fin{m}")
                nc.scalar.activation(fin[:], pss[:], AF.Identity,
                                     bias=kob_t[:, m:m + 1])
                nc.sync.dma_start(io["outT"][m * P:(m + 1) * P, :], fin[:])
            gemm_fm(io["kow"], D, D, yB, out_evict, pg2)


def _build():
    nc = bacc.Bacc("TRN2", debug=False, num_devices=NCORES)

    def din(name, shape, dt=BF16):
        return nc.dram_tensor(name, shape, dt, kind="ExternalInput").ap()

    io = {
        "xT": din("xT", [D, TOK]),
        "moew": din("moew", [D, D]),
        "qkw": din("qkw", [D, 2 * D]),
        "vw": din("vw", [D, D]),
        "vbb": din("vbb", [D]),
        "ow": din("ow", [D, D]),
        "f1w": din("f1w", [D, DFF]),
        "f2w": din("f2w", [DFF, D]),
        "fcw": din("fcw", [D, D]),
        "k1w": din("k1w", [D, D]),
        "kow": din("kow", [D, D]),
        "c_onesb": din("c_onesb", [1024], BF16),
        "c_ones": din("c_ones", [256], FP32),
        "c_eye": din("c_eye", [128, 128], FP32),
        "vrow": din("vrow", [H * (DH + 2)], FP32),
    }
    for name, shape in [("qkb16", [2 * D]), ("ob", [D]), ("f1b", [DFF]),
                        ("f2b", [D]), ("ln1g", [D]), ("ln1b", [D]),
                        ("ln2g", [D]), ("ln2b", [D]), ("fcb", [D]),
                        ("k1b", [D]), ("kob", [D]), ("moeb", [D])]:
        io[name] = din(name, shape, FP32)
    io["outT"] = nc.dram_tensor("outT", [D, TOK], FP32, kind="ExternalOutput").ap()

    with nc.allow_low_precision("bf16/fp8 matmul pipeline"):
        with tile.TileContext(nc) as tc:
            _body(nc, tc, io)
    nc.compile()
    return nc


# ----------------------------------------------------------------------------
# host side
# ----------------------------------------------------------------------------

def kernel(x, gw, gb, ew, eb, qkvw, qkvb, ow, ob, ln1g, ln1b, ln2g, ln2b,
           f1w, f1b, f2w, f2b, ffw, ffb, cfw, cfb, k1w, k1b, k2w, k2b,
           outw, outb):
    f64 = np.float64
    bf16 = ml_dtypes.bfloat16
    x = np.asarray(x, np.float32)
    gw, gb = np.asarray(gw, np.float32), np.asarray(gb, np.float32)
    ew, eb = np.asarray(ew, np.float32), np.asarray(eb, np.float32)
    qkvw, qkvb = np.asarray(qkvw, np.float32), np.asarray(qkvb, np.float32)

    # degenerate routing (token 0) + MoE layer fusion, all in f64
    x0 = x[0].astype(f64)
    Ws, bs = [], []
    for l in range(L):
        s = x0 @ gw[l].astype(f64) + gb[l].astype(f64)
        sel = np.argsort(-s, kind="stable")[:2]
        W = (ew[l][sel[0]].astype(f64) + ew[l][sel[1]].astype(f64)) * 0.5
        b = (eb[l][sel[0]].astype(f64) + eb[l][sel[1]].astype(f64)) * 0.5
        x0 = x0 @ W + b
        Ws.append(W)
        bs.append(b)
    Wf = Ws[0] @ Ws[1] @ Ws[2]
    bf_ = bs[0] @ Ws[1] @ Ws[2] + bs[1] @ Ws[2] + bs[2]

    # exact column sums of v for the attention uniform part
    vw_ = qkvw[:, 2 * D:].astype(f64)
    vb_ = qkvb[2 * D:].astype(f64)
    colx3 = x.astype(f64).sum(0) @ Wf + N * bf_
    vsum = colx3 @ vw_ + N * vb_                       # [D]
    vrow = np.zeros((H, DH + 2), np.float32)
    for h in range(H):
        vrow[h, :DH] = (2048.0 * vsum[h * DH:(h + 1) * DH]).astype(np.float32)
        vrow[h, DH] = 128.0 * N
    # fused trailing weights
    Wfc = np.asarray(ffw, f64) @ np.asarray(cfw, f64)
    bfc = np.asarray(ffb, f64) @ np.asarray(cfw, f64) + np.asarray(cfb, f64)
    Wko = np.asarray(k2w, f64) @ np.asarray(outw, f64)
    bko = np.asarray(k2b, f64) @ np.asarray(outw, f64) + np.asarray(outb, f64)

    if "nc" not in _CACHE:
        _CACHE["nc"] = _build()
    nc = _CACHE["nc"]

    shared = {
        "moew": Wf.astype(bf16), "moeb": bf_.astype(np.float32),
        "qkw": np.ascontiguousarray(qkvw[:, :2 * D]).astype(bf16),
        "qkb16": (qkvb[:2 * D] * 16.0).astype(np.float32),
        "vw": np.ascontiguousarray(vw_).astype(bf16),
        "vbb": vb_.astype(bf16),
        "ow": (np.asarray(ow, np.float32) / 16.0).astype(bf16),
        "ob": np.asarray(ob, np.float32),
        "f1w": np.asarray(f1w, np.float32).astype(bf16),
        "f1b": np.asarray(f1b, np.float32),
        "f2w": np.asarray(f2w, np.float32).astype(bf16),
        "f2b": np.asarray(f2b, np.float32),
        "ln1g": np.asarray(ln1g, np.float32), "ln1b": np.asarray(ln1b, np.float32),
        "ln2g": np.asarray(ln2g, np.float32), "ln2b": np.asarray(ln2b, np.float32),
        "fcw": Wfc.astype(bf16), "fcb": bfc.astype(np.float32),
        "k1w": np.asarray(k1w, np.float32).astype(bf16),
        "k1b": np.asarray(k1b, np.float32),
        "kow": Wko.astype(bf16), "kob": bko.astype(np.float32),
        "c_onesb": np.ones(1024, bf16),
        "c_ones": np.ones(256, np.float32),
        "c_eye": np.eye(128, dtype=np.float32),
        "vrow": vrow.reshape(-1),
    }

    in_maps = []
    for c in range(NCORES):
        m = dict(shared)
        m["xT"] = np.ascontiguousarray(x[c * TOK:(c + 1) * TOK].T).astype(bf16)
        in_maps.append(m)

    _CACHE["in_maps"] = in_maps
    res = bass_utils.run_bass_kernel_spmd(nc, in_maps, core_ids=list(range(NCORES)))
    _CACHE["last_result"] = res

    out = np.empty((N, D), np.float32)
    for c in range(NCORES):
        out[c * TOK:(c + 1) * TOK, :] = res.results[c]["outT"].T
    return out
